# revision 20
# baseline (speedup 1.0000x reference)
"""MoE (brute-force reference) kernel for 8 TRN2 NeuronCores.

Strategy: expert-parallel. Host routes token-slots by gate_idx to their
expert, pads each expert's slot list to capacity C, and transposes so the
device sees xt[e] = X_e.T [D, C]. Each core owns 2 experts and computes
  hT = gelu(w1 @ X)   (PSUM fp32, bf16 operands)
  yT = w2.T-free form  [D, C]
Weights stream from HBM as bf16 (halves DMA); accumulation is fp32 in
PSUM. b1 is applied on-device (per-partition bias in the gelu
activation); b2 and the gate_score combine happen on host (exact fp32).
"""

import numpy as np

import concourse.bacc as bacc
import concourse.mybir as mybir
from concourse import tile
from concourse.bass_utils import run_bass_kernel_spmd

E, D, H, TOPK, T = 16, 1024, 2048, 2, 2048
NCORES = 8
EPC = E // NCORES  # experts per core
C = 276            # per-expert token-slot capacity (seed-0 max is 275)
KD, KH, MD = D // 128, H // 128, D // 128  # 8, 16, 8

_F16 = np.float16
_CACHE: dict = {}


def _build(reps: int = 1):
    dt = mybir.dt.float16
    f32 = mybir.dt.float32
    nc = bacc.Bacc("TRN2", target_bir_lowering=False, debug=False,
                   num_devices=NCORES)
    xt = nc.dram_tensor("xt", [EPC, D, C], dt, kind="ExternalInput")
    w1t = nc.dram_tensor("w1t", [EPC, D, H], dt, kind="ExternalInput")
    w2t = nc.dram_tensor("w2t", [EPC, H, D], dt, kind="ExternalInput")
    b1 = nc.dram_tensor("b1", [EPC, 128, KH], f32, kind="ExternalInput")
    yt = nc.dram_tensor("yt", [EPC, D, C], dt, kind="ExternalOutput")

    gelu = mybir.ActivationFunctionType.Gelu_apprx_tanh
    MGRP = 8   # GEMM1 m-tiles per psum group (k-inner within a group)
    YGRP = 1   # y m-tiles per merged output DMA

    with tile.TileContext(nc) as tc:
        with (
            tc.tile_pool(name="xtp", bufs=2) as xtp,
            tc.tile_pool(name="w1p", bufs=2) as w1p,
            tc.tile_pool(name="w2p", bufs=2) as w2p,
            tc.tile_pool(name="htp", bufs=2) as htp,
            tc.tile_pool(name="yp", bufs=6) as yp,
            tc.tile_pool(name="bp", bufs=2) as bp,
            tc.tile_pool(name="ps", bufs=1, space="PSUM") as psp,
        ):
            # PE warm-up: tiny back-to-back matmuls while the first weight
            # DMAs stream in, so the HAM clock gate is at 8/8 when the real
            # matmul stream starts.
            zt = bp.tile([128, 128], dt, name="warmz", tag="warmz")
            nc.any.memset(zt[:], 0.0)
            psw = psp.tile([128, 128], f32, name="psw", tag="ps7")
            for _ in range(40):
                nc.tensor.matmul(psw[:], zt[:], zt[:],
                                 start=True, stop=True)

            for r in range(reps):
                for e in range(EPC):
                    u = f"{r}_{e}"
                    # X_e.T [D, C] -> [128, KD*C]: half 1 first, half 2 after
                    # the first-needed w1 halves.
                    # xt + b1 issue from gpsimd (SWDGE) so they don't sit on
                    # the HWDGE issue chain ahead of the weight DMAs.
                    xin = xt.ap()[e].rearrange("(k p) c -> p k c", p=128)
                    hk = KD // 2
                    xth = [xtp.tile([128, hk * C], dt, name=f"xt{u}_{i}",
                                    tag=f"xt{i}") for i in range(2)]

                    dma_eng = nc.gpsimd

                    def xt_dma(half):
                        dma_eng.dma_start(
                            out=xth[half][:].rearrange("p (k c) -> p k c",
                                                       k=hk),
                            in_=xin[:, half * hk:(half + 1) * hk, :])

                    def xtv(k):
                        return xth[k // hk][:, (k % hk) * C:(k % hk + 1) * C]

                    # w1 split into column halves: A = m-tiles 0..7, B =
                    # 8..15, streamed as merged k-chunks (fewer HWDGE
                    # issues). A chunks [1, 3, 4] so GEMM1 group 0 starts
                    # as soon as xt half 1 + the first slab land.
                    HH = H // 2

                    def wchunks(pool, pfx, dram, col0, ncol, sizes):
                        tiles, k0 = [], 0
                        for ci, n in enumerate(sizes):
                            tl = pool.tile([128, n * ncol], dt,
                                           name=f"{pfx}{u}_{ci}",
                                           tag=f"{pfx}{ci}")
                            nc.sync.dma_start(
                                out=tl[:].rearrange("p (k m) -> p k m", k=n),
                                in_=dram.ap()[e, k0 * 128:(k0 + n) * 128,
                                              col0:col0 + ncol]
                                    .rearrange("(k p) m -> p k m", p=128))
                            for j in range(n):
                                tiles.append(tl[:, j * ncol:(j + 1) * ncol])
                            k0 += n
                        return tiles

                    xt_dma(0)
                    w1a = wchunks(w1p, "w1a", w1t, 0, HH, [1] * KD)
                    xt_dma(1)
                    b1s = bp.tile([128, KH], f32, name=f"b1s{u}", tag="b1s")
                    dma_eng.dma_start(out=b1s[:], in_=b1.ap()[e])
                    w1b = wchunks(w1p, "w1b", w1t, HH, HH, [4, 4])
                    w2s = wchunks(w2p, "w2s", w2t, 0, D, [4, 4, 4, 4])

                    # GEMM1: hT[m] = gelu(sum_k w1s[k][:,m].T @ xts[k] + b1)
                    hts = [htp.tile([128, C], dt, name=f"ht{u}_{m}",
                                    tag=f"ht{m}") for m in range(KH)]
                    for g in range(0, KH, MGRP):
                        w1h = w1a if g == 0 else w1b
                        pss = [psp.tile([128, C], f32, name=f"ps1_{u}_{m}",
                                        tag=f"ps{m - g}")
                               for m in range(g, g + MGRP)]
                        for k in range(KD):
                            for i, m in enumerate(range(g, g + MGRP)):
                                mm = m - g
                                nc.tensor.matmul(
                                    pss[i][:],
                                    w1h[k][:, mm * 128:(mm + 1) * 128],
                                    xtv(k),
                                    start=(k == 0), stop=(k == KD - 1))
                        for i, m in enumerate(range(g, g + MGRP)):
                            nc.scalar.activation(
                                hts[m][:], pss[i][:], gelu,
                                bias=b1s[:, m:m + 1])

                    # GEMM2: yT[m] = sum_k w2s[k][:,m].T @ hts[k]
                    # k-inner per single m so evictions stream; merged out-DMA
                    ytv = yt.ap()[e].rearrange("(g p) c -> p g c", p=128)
                    for g in range(0, MD, YGRP):
                        yo = yp.tile([128, YGRP * C], dt, name=f"y{u}_{g}",
                                     tag="y")
                        for i, m in enumerate(range(g, g + YGRP)):
                            ps = psp.tile([128, C], f32, name=f"ps2_{u}_{m}",
                                          tag=f"ps{m % MGRP}")
                            for k in range(KH):
                                nc.tensor.matmul(
                                    ps[:],
                                    w2s[k][:, m * 128:(m + 1) * 128],
                                    hts[k][:],
                                    start=(k == 0), stop=(k == KH - 1))
                            nc.scalar.copy(out=yo[:, i * C:(i + 1) * C],
                                           in_=ps[:])
                        nc.gpsimd.dma_start(
                            out=ytv[:, g:g + YGRP, :],
                            in_=yo[:].rearrange("p (g c) -> p g c", g=YGRP))
    nc.compile()
    return nc


def _get_nc(reps: int = 1):
    if reps not in _CACHE:
        _CACHE[reps] = _build(reps)
    return _CACHE[reps]


def _route(inp, gate_idx, gate_score):
    """Returns (slot_ids per expert used, overflow slot ids per expert)."""
    g = np.asarray(gate_idx).astype(np.int64).reshape(-1)
    used, overflow = [], []
    for e in range(E):
        s = np.flatnonzero(g == e)
        used.append(s[:C])
        overflow.append(s[C:])
    return used, overflow


def kernel(inp, gate_idx, gate_score, w1, b1, w2, b2):
    inp = np.asarray(inp, dtype=np.float32)
    gate_idx = np.asarray(gate_idx)
    gate_score = np.asarray(gate_score, dtype=np.float32)
    w1 = np.asarray(w1, dtype=np.float32)
    b1 = np.asarray(b1, dtype=np.float32)
    w2 = np.asarray(w2, dtype=np.float32)
    b2 = np.asarray(b2, dtype=np.float32)

    used, overflow = _route(inp, gate_idx, gate_score)

    # Host-side gather + transpose, cast to bf16 for the device.
    xt_all = np.zeros((E, D, C), dtype=_F16)
    for e in range(E):
        toks = used[e] // TOPK
        if len(toks):
            xt_all[e, :, :len(toks)] = inp[toks].T.astype(_F16)
    w1t_all = np.ascontiguousarray(
        w1.transpose(0, 2, 1)).astype(_F16)  # [E, D, H]
    w2t_all = np.ascontiguousarray(
        w2.transpose(0, 2, 1)).astype(_F16)  # [E, H, D]

    in_maps = []
    for c in range(NCORES):
        sl = slice(EPC * c, EPC * (c + 1))
        in_maps.append({
            "xt": xt_all[sl],
            "w1t": w1t_all[sl],
            "w2t": w2t_all[sl],
            "b1": np.ascontiguousarray(
                b1[sl].reshape(EPC, KH, 128).transpose(0, 2, 1)),
        })

    nc = _get_nc()
    res = run_bass_kernel_spmd(nc, in_maps, list(range(NCORES)))

    # Host combine: scatter yT columns back to slots, weight by gate_score,
    # add the b2 term (folded out of the device kernel).
    y_all = np.zeros((T * TOPK, D), dtype=np.float32)
    for e in range(E):
        core, le = divmod(e, EPC)
        cnt = len(used[e])
        if cnt:
            y_all[used[e]] = res.results[core]["yt"][le, :, :cnt].T
        if len(overflow[e]):  # exact host fallback, never hit in practice
            x = inp[overflow[e] // TOPK]
            hh = x @ w1[e].T + b1[e]
            hh = 0.5 * hh * (1.0 + np.tanh(
                np.sqrt(2.0 / np.pi) * (hh + 0.044715 * hh ** 3)))
            y_all[overflow[e]] = hh @ w2[e].T

    out = np.einsum("tk,tkd->td", gate_score,
                    y_all.reshape(T, TOPK, D)).astype(np.float32)
    out += np.einsum("tk,tkd->td", gate_score,
                     b2[np.asarray(gate_idx).astype(np.int64)])
    return out
